# revision 1
# baseline (speedup 1.0000x reference)
"""Trainium2 Bass kernel for a Transformer-XL (MemTransformerLM) layer.

Sharding over 8 cores: core c = (b = c//4, head-group g = c%4 of 4 heads).
Each core computes its 4 heads' attention for its batch, a partial
attn_out = vec @ W_o[:, heads].T, then a ReduceScatter(+) over the quad
[[0..3],[4..7]] scatters query rows -> each core does LN1+FF+LN2 on its
256 rows. Host reassembles [1024, 2, 1024].

rel_shift: B = q_tilde @ rk^T is written per head to DRAM with row
stride 2304; BD[i,j] = B[i, j-i+1023] is read back with an oblique AP
(offset 1023, strides [2303, 1]) and PE-transpose-accumulated into the
AC^T PSUM group, so S^T = AC^T + BD^T lands in PSUM with no extra
vector pass. Mask = one triangular -1e30 addend tile on the band
kt == qt+8; key tiles kt > qt+8 are skipped entirely. Softmax skips the
max-subtraction (scores are O(1) by construction); the denominator
comes free from a ones-column appended to v.

DMA discipline: HWDGE trigger cost (~0.6 us/instruction) is the main
serial resource, so transfers are batched: full-row B writes, 512-wide
oblique reads, resident FF weights, packed small constants.
"""
import functools
import numpy as np

QLEN, MLEN, BSZ = 1024, 1024, 2
KLEN = QLEN + MLEN
D, H, DH, DI = 1024, 16, 64, 4096
HPG = 4                      # heads per group (per core)
HD_G = HPG * DH              # 256
N_CORES = 8
SCALE = 1.0 / (DH ** 0.5)
NEG = -1e30
BW = 2176                    # padded DRAM width for B (>= 2175)
NQT = QLEN // 128            # 8 query tiles of 128
NKT = KLEN // 128            # 16 key tiles of 128
NDC = D // 128               # 8 d-chunks
NMI = DI // 128              # 32 inner tiles
ROWS = QLEN // 4             # 256 rows per core after RS


@functools.lru_cache(maxsize=2)
def _build(single_sim=False):
    import concourse.bacc as bacc
    import concourse.mybir as mybir
    import concourse.tile as tile
    from concourse import masks
    import bass_rust

    F32 = mybir.dt.float32
    BF16 = mybir.dt.bfloat16
    AF = mybir.ActivationFunctionType
    ALU = mybir.AluOpType

    nc = bacc.Bacc("TRN2", target_bir_lowering=False, debug=False,
                   num_devices=N_CORES)

    def din(name, shape, dt=F32):
        return nc.dram_tensor(name, shape, dt, kind="ExternalInput")

    cat_fm = din("cat_fm", [D, KLEN], BF16)   # [d, mems||w tokens], this b
    r_fm = din("r_fm", [D, KLEN], BF16)       # r transposed
    wpk = din("wpk", [D, 4 * HD_G], BF16)     # [wkT | wqT | wrT | wvT]
    biases = din("biases", [128, 8])          # raw rwb/rrb + pre-scaled
    woT = din("woT", [HD_G, D], BF16)         # W_o^T rows for group
    w1T = din("w1T", [D, DI], BF16)
    b1c = din("b1c", [128, NMI])              # b1 packed column-wise
    w2T = din("w2T", [DI, D], BF16)
    b2r = din("b2r", [128, D])                # broadcast rows
    b2s = din("b2s", [1, D], BF16)            # b2 single row, bf16
    g1r = din("g1r", [128, D])
    bb1r = din("bb1r", [128, D])
    g2r = din("g2r", [128, D])
    bb2r = din("bb2r", [128, D])
    wres = din("wres", [ROWS, D])             # w rows for residual (b, Q slice)
    tri = din("tri", [128, 128])              # -1e30 where jj > ii else 0

    Bh = [nc.dram_tensor(f"Bh{h}", [QLEN * BW], BF16) for h in range(HPG)]
    if single_sim:
        attn_part = nc.dram_tensor("attn_part", [QLEN, D], BF16,
                                   kind="ExternalOutput")
    else:
        attn_part = nc.dram_tensor("attn_part", [QLEN, D], BF16)
    rs_out = nc.dram_tensor("rs_out", [ROWS, D], BF16)
    y = nc.dram_tensor("y", [ROWS, D], F32, kind="ExternalOutput")

    def obl(h, qt, kt, w):
        # oblique view of Bh[h]: BD tile [128 q, w keys] at (qt, 128*kt)
        off = 1023 + 128 * qt * (BW - 1) + 128 * kt
        return bass_rust.AP(tensor=Bh[h].ap().tensor, offset=off,
                            ap=[[BW - 1, 128], [1, w]])

    def bwrite(h, qt, c0, w):
        # B row tile [128, w] at (row 128*qt, col c0)
        off = 128 * qt * BW + c0
        return bass_rust.AP(tensor=Bh[h].ap().tensor, offset=off,
                            ap=[[BW, 128], [1, w]])

    def bpad(h):
        # all pad columns of head h as one flat write matching [128, 2048] src
        off = 2048
        return bass_rust.AP(tensor=Bh[h].ap().tensor, offset=off,
                            ap=[[128 * BW, NQT], [BW, 128], [1, BW - 2048]])

    with tile.TileContext(nc) as tc:
        with tc.tile_pool(name="const", bufs=1) as cpool, \
             tc.tile_pool(name="work", bufs=2) as wpool, \
             tc.tile_pool(name="psA", bufs=3, space="PSUM") as psA, \
             tc.tile_pool(name="psB", bufs=3, space="PSUM") as psB, \
             tc.tile_pool(name="psV", bufs=1, space="PSUM") as psV, \
             tc.tile_pool(name="psT", bufs=1, space="PSUM") as psT:

            # ---------------- global constants ----------------
            identb = cpool.tile([128, 128], BF16, tag="identb")
            masks.make_identity(nc, identb[:])
            ident = cpool.tile([128, 128], F32, tag="ident")
            masks.make_identity(nc, ident[:])
            bias_t = cpool.tile([128, 8], F32, tag="bias")
            nc.scalar.dma_start(out=bias_t[:], in_=biases[:])
            b1c_t = cpool.tile([128, NMI], F32, tag="b1c")
            nc.scalar.dma_start(out=b1c_t[:], in_=b1c[:])
            # out1 survives attention scope into FF scope
            out1 = [cpool.tile([128, D], F32, tag=f"out1_{t}", name=f"out1_{t}")
                    for t in range(ROWS // 128)]

            onesr = cpool.tile([1, 128], BF16, tag="onesr")
            nc.vector.memset(onesr[:], 1.0)
            b2b = cpool.tile([1, D], BF16, tag="b2b")
            nc.scalar.dma_start(out=b2b[:], in_=b2s[:])
            zpad = cpool.tile([128, (BW - 2048) * NQT], BF16, tag="zpad")
            nc.vector.memset(zpad[:], NEG)
            for h in range(HPG):
                nc.scalar.dma_start(out=bpad(h), in_=zpad[:])

            # ================ attention scope ================
            with tc.tile_pool(name="attn", bufs=1) as apool, \
                 tc.tile_pool(name="prob", bufs=2) as ppool:

                # ---------------- P1: inputs then projection weights ------
                cat_t = []
                pw = []
                for k in range(NDC):
                    tt = apool.tile([128, KLEN], BF16, tag=f"cat{k}",
                                    name=f"cat{k}")
                    nc.sync.dma_start(out=tt[:], in_=cat_fm[128 * k:128 * k + 128, :])
                    cat_t.append(tt)
                    tt = apool.tile([128, 4 * HD_G], BF16, tag=f"pw{k}",
                                    name=f"pw{k}")
                    nc.sync.dma_start(out=tt[:], in_=wpk[128 * k:128 * k + 128, :])
                    pw.append(tt)

                def pw_sl(k, which, m):
                    base = {"wkT": 0, "wqT": 1, "wrT": 2, "wvT": 3}[which] * HD_G
                    return pw[k][:, base + 128 * m:base + 128 * m + 128]

                woT_t = []
                for k in range(HD_G // 128):
                    tt = apool.tile([128, D], BF16, tag=f"woT{k}", name=f"woT{k}")
                    nc.sync.dma_start(out=tt[:], in_=woT[128 * k:128 * k + 128, :])
                    woT_t.append(tt)
                # r shares slots with cat (tag reuse, written after cat released)
                r_t = []
                for k in range(NDC):
                    tt = apool.tile([128, KLEN], BF16, tag=f"r{k}",
                                    name=f"r{k}")
                    nc.sync.dma_start(out=tt[:], in_=r_fm[128 * k:128 * k + 128, :])
                    r_t.append(tt)

                k_fm, rk_fm, qh_fm, qt_fm = [], [], [], []
                for m in range(2):
                    k_fm.append(apool.tile([128, KLEN], BF16, tag=f"kfm{m}",
                                           name=f"kfm{m}"))
                    rk_fm.append(apool.tile([128, KLEN], BF16, tag=f"rkfm{m}",
                                            name=f"rkfm{m}"))
                    qh_fm.append(apool.tile([128, QLEN], BF16, tag=f"qhfm{m}",
                                            name=f"qhfm{m}"))
                    qt_fm.append(apool.tile([128, QLEN], BF16, tag=f"qtfm{m}",
                                            name=f"qtfm{m}"))
                for m in range(2):
                    for n in range(KLEN // 512):
                        ps = psA.tile([128, 512], F32, tag="psA", name="psk")
                        for k in range(NDC):
                            nc.tensor.matmul(
                                ps[:], pw_sl(k, "wkT", m),
                                cat_t[k][:, 512 * n:512 * n + 512],
                                start=(k == 0), stop=(k == NDC - 1))
                        nc.scalar.activation(k_fm[m][:, 512 * n:512 * n + 512],
                                             ps[:], AF.Copy)
                    for n in range(QLEN // 512):
                        ps = psA.tile([128, 512], F32, tag="psA", name="psq")
                        for k in range(NDC):
                            nc.tensor.matmul(
                                ps[:], pw_sl(k, "wqT", m),
                                cat_t[k][:, MLEN + 512 * n:MLEN + 512 * n + 512],
                                start=(k == 0), stop=(k == NDC - 1))
                        # (q + bias) * SCALE on DVE, cast to bf16
                        nc.vector.tensor_scalar(
                            out=qh_fm[m][:, 512 * n:512 * n + 512], in0=ps[:],
                            scalar1=bias_t[:, m:m + 1], scalar2=SCALE,
                            op0=ALU.add, op1=ALU.mult)
                        nc.scalar.activation(
                            qt_fm[m][:, 512 * n:512 * n + 512], ps[:],
                            AF.Identity, scale=SCALE,
                            bias=bias_t[:, 6 + m:7 + m])
                # v token-major with interleaved ones cols: [128, 260] per kt
                v_tok = []
                for kt in range(NKT):
                    vt = apool.tile([128, 65 * HPG], BF16, tag=f"vtok{kt}",
                                    name=f"vtok{kt}")
                    ps = psB.tile([128, HD_G], F32, tag="psB", name="psv")
                    for k in range(NDC):
                        nc.tensor.matmul(
                            ps[:], cat_t[k][:, 128 * kt:128 * kt + 128],
                            pw[k][:, 3 * HD_G:4 * HD_G],
                            start=(k == 0), stop=(k == NDC - 1))
                    for h in range(HPG):
                        nc.scalar.activation(vt[:, 65 * h:65 * h + 64],
                                             ps[:, 64 * h:64 * h + 64], AF.Copy)
                        nc.vector.memset(vt[:, 65 * h + 64:65 * h + 65], 1.0)
                    v_tok.append(vt)
                # rk projection (r shares cat slots, runs after v frees them)
                for m in range(2):
                    for n in range(KLEN // 512):
                        ps = psA.tile([128, 512], F32, tag="psA", name="psr")
                        for k in range(NDC):
                            nc.tensor.matmul(
                                ps[:], pw_sl(k, "wrT", m),
                                r_t[k][:, 512 * n:512 * n + 512],
                                start=(k == 0), stop=(k == NDC - 1))
                        nc.scalar.activation(rk_fm[m][:, 512 * n:512 * n + 512],
                                             ps[:], AF.Copy)

                # ---------------- P2: attention per head ----------------
                vecT_fm = {}
                for m in range(2):
                    for hf in range(2):
                        vecT_fm[(m, hf)] = apool.tile(
                            [128, QLEN // 2], BF16, tag=f"vecT{m}_{hf}",
                            name=f"vecT{m}_{hf}")
                for h in range(HPG):
                    m, p0 = h // 2, 64 * (h % 2)
                    qh_h = qh_fm[m][p0:p0 + 64, :]
                    qt_h = qt_fm[m][p0:p0 + 64, :]
                    k_h = k_fm[m][p0:p0 + 64, :]
                    rk_h = rk_fm[m][p0:p0 + 64, :]

                    # B = q_tilde @ rk^T -> DRAM bf16 rows; for qt<=3 the
                    # leading 512 columns are never read back -> skip them.
                    for qt in range(NQT):
                        ct0 = 1 if qt <= 3 else 0
                        bs = wpool.tile([128, KLEN], BF16, tag="st2048",
                                        bufs=3, name="bs")
                        for ct in range(ct0, KLEN // 512):
                            ps = psB.tile([128, 512], F32, tag="psB", name="psb")
                            nc.tensor.matmul(
                                ps[:], qt_h[:, 128 * qt:128 * qt + 128],
                                rk_h[:, 512 * ct:512 * ct + 512],
                                start=True, stop=True)
                            nc.vector.tensor_copy(bs[:, 512 * ct:512 * ct + 512],
                                                  ps[:])
                        nc.sync.dma_start(
                            out=bwrite(h, qt, 512 * ct0, KLEN - 512 * ct0),
                            in_=bs[:, 512 * ct0:])

                    for qh2 in range(2):       # q halves of 512
                        probT = [ppool.tile([128, 512], BF16, tag=f"pT{kt}",
                                            name=f"pT{kt}_{h}_{qh2}")
                                 for kt in range(NKT)]
                        # batched oblique BD reads: [128, <=512] per (qt, ktb),
                        # loaded in consumption (ktb-major) order
                        bd_tiles = {}
                        for ktb in range(4):
                            for qt in range(4 * qh2, 4 * qh2 + 4):
                                kmax = min(qt + 8, NKT - 1)
                                if 4 * ktb > kmax:
                                    continue
                                wdt = min(512, (kmax + 1 - 4 * ktb) * 128)
                                bd16 = wpool.tile([128, 512], BF16, tag="bd16",
                                                  bufs=4, name=f"bd16_{qt}{ktb}")
                                nc.sync.dma_start(out=bd16[:, 0:wdt],
                                                  in_=obl(h, qt, 4 * ktb, wdt))
                                bd = wpool.tile([128, 512], F32, tag="bd",
                                                bufs=8, name=f"bd{qt}_{ktb}")
                                nc.gpsimd.tensor_copy(bd[:, 0:wdt],
                                                      bd16[:, 0:wdt])
                                bd_tiles[(qt, ktb)] = bd
                        for kt in range(NKT):
                            qts = [qt for qt in range(4 * qh2, 4 * qh2 + 4)
                                   if qt >= kt - 8]
                            if not qts:
                                continue
                            ps = psA.tile([128, 512], F32, tag="psA", name="pss")
                            nc.tensor.matmul(
                                ps[:], k_h[:, 128 * kt:128 * kt + 128],
                                qh_h[:, 512 * qh2:512 * qh2 + 512],
                                start=True, stop=False)
                            for i, qt in enumerate(qts):
                                bd = bd_tiles[(qt, kt // 4)]
                                bo = 128 * (kt % 4)
                                sub = 128 * (qt - 4 * qh2)
                                nc.tensor.matmul(ps[:, sub:sub + 128],
                                                 bd[:, bo:bo + 128],
                                                 ident[:], is_transpose=True,
                                                 start=False,
                                                 stop=(i == len(qts) - 1))
                            blo, bhi = qts[0], 4 * qh2 + 4
                            sub = 128 * (blo - 4 * qh2)
                            w = 128 * (bhi - blo)
                            nc.scalar.activation(
                                probT[kt][:, sub:sub + w],
                                ps[:, sub:sub + w], AF.Exp)

                        # vec per query tile in this half
                        for qt in range(4 * qh2, 4 * qh2 + 4):
                            kmax = min(qt + 8, NKT - 1)
                            pv = psV.tile([128, 65], F32, tag="psV", name="pv")
                            sub = 128 * (qt - 4 * qh2)
                            for kt in range(kmax + 1):
                                nc.tensor.matmul(
                                    pv[:], probT[kt][:, sub:sub + 128],
                                    v_tok[kt][:, 65 * h:65 * h + 65],
                                    start=(kt == 0), stop=(kt == kmax))
                            rec = wpool.tile([128, 1], F32, tag="rec", name="rec")
                            nc.vector.reciprocal(rec[:], pv[:, 64:65])
                            vn = wpool.tile([128, 64], BF16, tag="vn", name="vn")
                            nc.scalar.activation(vn[:], pv[:, 0:64], AF.Identity,
                                                 scale=rec[:])
                            pt = psT.tile([64, 128], BF16, tag="psT", name="ptr")
                            nc.tensor.matmul(pt[:], vn[:], identb[:],
                                             is_transpose=True,
                                             start=True, stop=True)
                            nc.scalar.activation(
                                vecT_fm[(m, qh2)][p0:p0 + 64,
                                                  128 * (qt % 4):128 * (qt % 4) + 128],
                                pt[:], AF.Copy)

                # ---------------- P3: partial attn_out ----------------
                for qt in range(NQT):
                    ao = wpool.tile([128, D], BF16, tag="st2048", bufs=3,
                                    name="ao")
                    for n in range(D // 512):
                        ps = psA.tile([128, 512], F32, tag="psA", name="pso")
                        for k in range(2):
                            nc.tensor.matmul(
                                ps[:],
                                vecT_fm[(k, qt // 4)][:, 128 * (qt % 4):
                                                      128 * (qt % 4) + 128],
                                woT_t[k][:, 512 * n:512 * n + 512],
                                start=(k == 0), stop=(k == 1))
                        nc.vector.tensor_copy(ao[:, 512 * n:512 * n + 512],
                                              ps[:])
                    nc.sync.dma_start(
                        out=attn_part[128 * qt:128 * qt + 128, :], in_=ao[:])

            # ---------------- P4: ReduceScatter over quads, 2 halves ------
            # half s covers query rows [512s, 512s+512); core rank rr gets
            # global rows [512s + 128rr, +128) -> rs_out[128s : 128s+128].
            for s in range(2):
                if single_sim:
                    nc.sync.dma_start(
                        out=rs_out[128 * s:128 * s + 128, :],
                        in_=attn_part[512 * s:512 * s + 128, :])
                else:
                    nc.gpsimd.collective_compute(
                        "ReduceScatter", ALU.add,
                        replica_groups=[[0, 1, 2, 3], [4, 5, 6, 7]],
                        ins=[attn_part[512 * s:512 * s + 512, :]],
                        outs=[rs_out[128 * s:128 * s + 128, :]])

            # ================ FF scope ================
            def layer_norm(x_t, g_row, b_row, out_t):
                # in place on x_t
                s = wpool.tile([128, 1], F32, tag="lns", name="lns")
                nc.vector.reduce_sum(s[:], x_t[:], axis=mybir.AxisListType.X)
                mn = wpool.tile([128, 1], F32, tag="lnm", name="lnm")
                nc.vector.tensor_scalar_mul(mn[:], s[:], 1.0 / D)
                nc.vector.tensor_scalar(out=x_t[:], in0=x_t[:], scalar1=mn[:],
                                        scalar2=None, op0=ALU.subtract)
                sq = fpool.tile([128, D], F32, tag="lnsq", bufs=2, name="lnsq")
                v2 = wpool.tile([128, 1], F32, tag="lnv", name="lnv")
                nc.vector.tensor_tensor(out=sq[:], in0=x_t[:], in1=x_t[:],
                                        op=ALU.mult)
                nc.vector.reduce_sum(v2[:], sq[:], axis=mybir.AxisListType.X)
                ve = wpool.tile([128, 1], F32, tag="lnve", name="lnve")
                nc.vector.tensor_scalar(out=ve[:], in0=v2[:], scalar1=1.0 / D,
                                        scalar2=1e-5, op0=ALU.mult, op1=ALU.add)
                rc = wpool.tile([128, 1], F32, tag="lnrc", name="lnrc")
                nc.vector.reciprocal(rc[:], ve[:])
                rstd = wpool.tile([128, 1], F32, tag="lnrstd", name="lnrstd")
                nc.scalar.activation(rstd[:], rc[:], AF.Sqrt)
                nc.vector.tensor_scalar_mul(x_t[:], x_t[:], rstd[:])
                nc.vector.tensor_tensor(out=x_t[:], in0=x_t[:], in1=g_row[:],
                                        op=ALU.mult)
                nc.vector.tensor_tensor(out=out_t[:], in0=x_t[:], in1=b_row[:],
                                        op=ALU.add)

            with tc.tile_pool(name="ff", bufs=1) as fpool, \
                 tc.tile_pool(name="w2s", bufs=4) as w2pool:

                rows = {}
                for nm, t in [("g1r", g1r), ("bb1r", bb1r),
                              ("g2r", g2r), ("bb2r", bb2r)]:
                    rt = fpool.tile([128, D], F32, tag=nm, name=nm)
                    nc.scalar.dma_start(out=rt[:], in_=t[:])
                    rows[nm] = rt

                # P5: residual + LN1
                for t in range(ROWS // 128):
                    rtb = fpool.tile([128, D], BF16, tag="rsx", bufs=2,
                                     name=f"rs{t}")
                    nc.sync.dma_start(out=rtb[:],
                                      in_=rs_out[128 * t:128 * t + 128, :])
                    wt = fpool.tile([128, D], F32, tag="wres", bufs=2, name=f"wres{t}")
                    nc.sync.dma_start(out=wt[:],
                                      in_=wres[128 * t:128 * t + 128, :])
                    nc.vector.tensor_tensor(out=wt[:], in0=wt[:], in1=rtb[:],
                                            op=ALU.add)
                    layer_norm(wt, rows["g1r"], rows["bb1r"], out1[t])

                # P6: FF — transpose out1 to feature-major
                out1_fm = []
                for k in range(NDC):
                    ofm = fpool.tile([128, ROWS], BF16, tag=f"o1fm{k}",
                                     name=f"o1fm{k}")
                    out1_fm.append(ofm)
                for t in range(ROWS // 128):
                    for k in range(NDC):
                        pt = psT.tile([128, 128], F32, tag="psT", name="ptf")
                        nc.tensor.matmul(pt[:], out1[t][:, 128 * k:128 * k + 128],
                                         ident[:], is_transpose=True,
                                         start=True, stop=True)
                        nc.scalar.activation(out1_fm[k][:, 128 * t:128 * t + 128],
                                             pt[:], AF.Copy)

                # FF1 + FF2 interleaved per mi
                hps = {}
                hps[(0, 0)] = psB.tile([128, 512], F32, tag="psB", name="h2ps00")
                hps[(0, 1)] = psB.tile([128, 512], F32, tag="psB", name="h2ps01")
                hps[(1, 0)] = psV.tile([128, 512], F32, tag="psV", name="h2ps10")
                hps[(1, 1)] = psT.tile([128, 512], F32, tag="psT", name="h2ps11")
                w1b = {}
                # inject b2 into each h2 group: out[p, j] += 1 * b2[j]
                for (t, n), hp in hps.items():
                    nc.tensor.matmul(hp[:], onesr[:, 0:128],
                                     b2b[:, 512 * n:512 * n + 512],
                                     start=True, stop=False)
                for mi in range(NMI):
                    if mi % 16 == 0:
                        # stream W1^T in 1024-wide mi-blocks (8 k-tiles each)
                        for k in range(NDC):
                            bt = fpool.tile([128, 2048], BF16, tag=f"w1b{k}",
                                            bufs=2, name=f"w1b{k}_{mi}")
                            nc.sync.dma_start(
                                out=bt[:],
                                in_=w1T[128 * k:128 * k + 128,
                                        128 * mi:128 * mi + 2048])
                            w1b[k] = bt
                    w2t = w2pool.tile([128, D], BF16, tag="w2", name=f"w2_{mi}")
                    nc.sync.dma_start(out=w2t[:],
                                      in_=w2T[128 * mi:128 * mi + 128, :])
                    mo = 128 * (mi % 16)
                    for t in range(ROWS // 128):
                        ps = psA.tile([128, 128], F32, tag="psA", name="psh1")
                        for k in range(NDC):
                            nc.tensor.matmul(
                                ps[:], w1b[k][:, mo:mo + 128],
                                out1_fm[k][:, 128 * t:128 * t + 128],
                                start=(k == 0), stop=(k == NDC - 1))
                        ht = fpool.tile([128, 128], BF16, tag=f"h1T{mi}_{t}",
                                        bufs=1, name=f"h1T{mi}_{t}")
                        nc.scalar.activation(ht[:], ps[:], AF.Relu,
                                             bias=b1c_t[:, mi:mi + 1])
                        for n in range(D // 512):
                            nc.tensor.matmul(
                                hps[(t, n)][:], ht[:],
                                w2t[:, 512 * n:512 * n + 512],
                                start=False, stop=(mi == NMI - 1))
                for t in range(ROWS // 128):
                    x2 = fpool.tile([128, D], F32, tag="wres", bufs=2, name=f"x2_{t}")
                    for n in range(D // 512):
                        nc.vector.tensor_tensor(
                            out=x2[:, 512 * n:512 * n + 512], in0=hps[(t, n)][:],
                            in1=out1[t][:, 512 * n:512 * n + 512], op=ALU.add)
                    yt = fpool.tile([128, D], F32, tag="lnsq", bufs=2, name=f"y_{t}")
                    layer_norm(x2, rows["g2r"], rows["bb2r"], yt)
                    nc.sync.dma_start(out=y[128 * t:128 * t + 128, :], in_=yt[:])

    nc.compile()
    return nc


def _prep_inputs(w, r, mems, W_qkv, W_r, W_o, r_w_bias, r_r_bias,
                 ln1_g, ln1_b, ff_W1, ff_b1, ff_W2, ff_b2, ln2_g, ln2_b,
                 attn_mask=None):
    import ml_dtypes
    f32 = np.float32
    bf16 = ml_dtypes.bfloat16
    cat = np.concatenate([mems, w], axis=0)            # [KLEN, B, D]
    cat_fm = [np.ascontiguousarray(cat[:, b, :].T).astype(bf16)
              for b in range(BSZ)]
    r_fm = np.ascontiguousarray(r.T).astype(bf16)
    w1T = np.ascontiguousarray(ff_W1.T).astype(bf16)   # [D, DI]
    w2T = np.ascontiguousarray(ff_W2.T).astype(bf16)   # [DI, D]
    woT_full = np.ascontiguousarray(W_o.T, dtype=f32)  # [H*DH, D]
    b1c = np.ascontiguousarray(
        np.asarray(ff_b1, f32).reshape(NMI, 128).T)    # [128, NMI]
    rowb = lambda v: np.ascontiguousarray(
        np.broadcast_to(np.asarray(v, f32).reshape(1, D), (128, D)))
    tri = np.where(np.arange(128)[:, None] > np.arange(128)[None, :],
                   np.float32(NEG), np.float32(0.0))
    tri = np.ascontiguousarray(tri, dtype=f32)

    in_maps = []
    for c in range(N_CORES):
        b, g = c // 4, c % 4
        sl = slice(HD_G * g, HD_G * g + HD_G)
        wkT = np.asarray(W_qkv, f32)[H * DH:2 * H * DH][sl].T
        wqT = np.asarray(W_qkv, f32)[0:H * DH][sl].T
        wrT = np.asarray(W_r, f32)[sl].T
        wvT = np.asarray(W_qkv, f32)[2 * H * DH:3 * H * DH][sl].T
        wpk = np.concatenate([wkT, wqT, wrT, wvT], axis=1)  # [D, 4*HD_G]
        rwbv = np.asarray(r_w_bias, f32).reshape(-1)[sl]
        rrbv = np.asarray(r_r_bias, f32).reshape(-1)[sl]
        bias = np.stack([
            rwbv[0:128], rwbv[128:256], rrbv[0:128], rrbv[128:256],
            rwbv[0:128] * SCALE, rwbv[128:256] * SCALE,
            rrbv[0:128] * SCALE, rrbv[128:256] * SCALE,
        ], axis=1)                                          # [128, 8]
        m = {
            "cat_fm": cat_fm[b],
            "r_fm": r_fm,
            "wpk": np.ascontiguousarray(wpk).astype(bf16),
            "biases": np.ascontiguousarray(bias),
            "woT": np.ascontiguousarray(woT_full[sl]).astype(bf16),
            "w1T": w1T, "b1c": b1c, "w2T": w2T,
            "b2r": rowb(ff_b2),
            "b2s": np.ascontiguousarray(
                np.asarray(ff_b2, f32).reshape(1, D)).astype(bf16), "g1r": rowb(ln1_g), "bb1r": rowb(ln1_b),
            "g2r": rowb(ln2_g), "bb2r": rowb(ln2_b),
            "wres": np.ascontiguousarray(np.concatenate(
                [np.asarray(w, f32)[128 * g:128 * g + 128, b, :],
                 np.asarray(w, f32)[512 + 128 * g:512 + 128 * g + 128, b, :]],
                axis=0)),
            "tri": tri,
        }
        in_maps.append(m)
    return in_maps


def kernel(**inputs):
    from concourse.bass_utils import run_bass_kernel_spmd
    nc = _build()
    in_maps = _prep_inputs(**{k: np.asarray(v) for k, v in inputs.items()})
    res = run_bass_kernel_spmd(nc, in_maps, list(range(N_CORES)))
    out = np.empty((QLEN, BSZ, D), np.float32)
    for c in range(N_CORES):
        b, g = c // 4, c % 4
        yv = res.results[c]["y"]
        out[128 * g:128 * g + 128, b, :] = yv[0:128]
        out[512 + 128 * g:512 + 128 * g + 128, b, :] = yv[128:256]
    return out

